# revision 24
# baseline (speedup 1.0000x reference)
"""GCNConv Bass kernel for Trainium2, 8 NeuronCores (axon).

Math (per reference):
    deg[n] = in-degree of n over col (incl. self-loops)
    dis[n] = rsqrt(deg[n])
    out    = D^-1/2 (A + I) D^-1/2 x W^T + b

Dense-streaming mixed fp16 x fp8 formulation (no gather):
    Host builds a dense per-core COUNT matrix B[s, d] (number of edges
    s->d, small ints, exact in fp8e4m3, [10240 x 1280] = 13.1 MB/core)
    and row-scaled features y = dis[s] * x[s] in fp16.  The PE streams
    B as the fp8 moving operand against stationary fp16 y tiles
    (mixed-dtype matmul, HW-validated exact):
        agg[f, d] = sum_s y[s, f] * B[s, d]
    Epilogue applies dis[d], projects through W^T (fp16), adds bias.

    vs the per-edge dma_gather baseline (1.2 ms, 96% DMA-bound on random
    256 B reads): streams 16 MB/core at ~430 GB/s; B is fully SBUF-
    resident so all DMA front-loads, PE runs back-to-back after.

Device pipeline per core (dest nodes c*1280 .. (c+1)*1280):
    1. warmup matmuls on a memset tile (no DMA dep; HAM clock-gate warm)
    2. bias pre-seeded into the projection PSUM via [1,128] matmuls
    3. B chunks stream into resident SBUF tiles (4 s-tiles = 655 KB per
       DMA, chunk-contiguous DRAM, double HWDGE rings, never recycled)
    4. PE: agg += y_t^T @ B_t per s-tile, 3 segments of 512/512/256 dest
       cols, segment order snaked to halve PSUM-bank transitions
    5. epilogue: agg16 = agg * disb (DVE, fused scale+cast), 10
       projection matmuls vs W^T, cast, one f16 DMA (host untiles+casts)
"""

import os
import sys
import types

import numpy as np

N_NODES = 10000
N_EDGES = 640000
C = 128
NCORES = 8
DPC = 1280               # dest nodes per core (10000 padded to 10240)
NST = 80                 # source tiles of 128 (10240 >= 10000)
NSP = NST * 128
NDB = DPC // 128         # 10 dest blocks per core
TPG = 4                  # s-tiles per B DMA chunk (655 KB fp8)
NCH = NST // TPG         # 20 chunks, all SBUF-resident
SEGS = ((0, 512), (512, 1024), (1024, 1280))
NWARM = 60

_cache = {}
last_exec_time_ns = None


def _install_ntff_shim():
    if "antenv.axon_hooks" in sys.modules:
        return
    mod = types.ModuleType("antenv.axon_hooks")
    mod._hook = None
    mod.set_axon_ntff_profile_hook = lambda h: setattr(mod, "_hook", h)
    mod.get_axon_ntff_profile_hook = lambda: mod._hook
    sys.modules["antenv.axon_hooks"] = mod
    try:
        import antenv
        antenv.axon_hooks = mod
        from trn_agent_boot.trn_boot import _ntff_profile_via_ctypes
        mod._hook = _ntff_profile_via_ctypes("/opt/axon/libaxon_pjrt.so")
    except Exception:
        pass


def _prep(x, edge_index):
    import ml_dtypes

    row = edge_index[0].astype(np.int64)
    col = edge_index[1].astype(np.int64)
    loops = np.arange(N_NODES, dtype=np.int64)
    row = np.concatenate([row, loops])
    col = np.concatenate([col, loops])
    deg = np.bincount(col, minlength=N_NODES).astype(np.float64)
    dis = np.where(deg > 0, 1.0 / np.sqrt(deg), 0.0)

    bs, dcols = [], []
    for c in range(NCORES):
        c0 = c * DPC
        m = (col >= c0) & (col < c0 + DPC)
        idx = row[m] * DPC + (col[m] - c0)
        B = np.bincount(idx, minlength=NSP * DPC).astype(ml_dtypes.float8_e4m3)
        # chunk-contiguous layout: chunk ch = rows [ch*128,(ch+1)*128),
        # columns (k, d); each chunk is a contiguous 655 KB DRAM block
        B = B.reshape(NCH, TPG, 128, DPC).transpose(0, 2, 1, 3)
        bs.append(np.ascontiguousarray(B.reshape(NCH * 128, TPG * DPC)))
        dcol = np.zeros(DPC, np.float64)
        hi = min(c0 + DPC, N_NODES)
        if hi > c0:
            dcol[: hi - c0] = dis[c0:hi]
        dcols.append(dcol)

    # y = dis_s * x, fp16, tiled [128, (t, f)]
    y = (dis[:, None] * x.astype(np.float64)).astype(np.float16)
    yp = np.zeros((NSP, C), np.float16)
    yp[:N_NODES] = y
    xt = np.ascontiguousarray(
        yp.reshape(NST, 128, C).transpose(1, 0, 2).reshape(128, NST * C))
    return xt, bs, dcols


def _build():
    import concourse.bacc as bacc
    import concourse.tile as tile
    from concourse import mybir

    f32 = mybir.dt.float32
    f16 = mybir.dt.float16
    f8 = mybir.dt.float8e4

    nc = bacc.Bacc("TRN2", target_bir_lowering=False)
    xt_in = nc.dram_tensor("xt", [128, NST * C], f16, kind="ExternalInput")
    b8_in = nc.dram_tensor("b8", [NCH * 128, TPG * DPC], f8,
                           kind="ExternalInput")
    # combined consts: [disb (1280) | wt (128) | b broadcast (128)]
    cst_in = nc.dram_tensor("cst", [128, DPC + 2 * C], f16,
                            kind="ExternalInput")
    out_t = nc.dram_tensor("out", [128, NDB * C], f16, kind="ExternalOutput")

    with tile.TileContext(nc) as tc:
        with (
            tc.tile_pool(name="const", bufs=1) as cp,
            tc.tile_pool(name="btp", bufs=1) as bp_,
            tc.tile_pool(name="epi", bufs=1) as ep,
            tc.tile_pool(name="psum", bufs=1, space="PSUM") as pp,
            tc.tile_pool(name="psum2", bufs=1, space="PSUM") as pp2,
            tc.tile_pool(name="psum3", bufs=1, space="PSUM") as pp3,
        ):
            # warmup weights via memset: no DMA dependency, PE can start
            # as soon as the engines boot
            wu_w = cp.tile([128, 128], f16)
            nc.vector.memset(wu_w[:], 0.25)
            wup = pp3.tile([128, 128], f32, space="PSUM")
            for _ in range(NWARM):
                nc.tensor.matmul(out=wup[:], lhsT=wu_w[:], rhs=wu_w[:],
                                 start=True, stop=True)

            cst = cp.tile([128, DPC + 2 * C], f16)
            disb_sb = cst[:, 0:DPC]
            wt_sb = cst[:, DPC : DPC + C]
            b_row = cst[0:1, DPC + C : DPC + 2 * C]
            ones1 = cp.tile([1, 128], f16)
            nc.vector.memset(ones1[:], 1.0)

            # x tiles in FOUR separate SBUF tiles (656 KB each) so the
            # stream starts once the first piece + first B chunk land
            XW = NST // 4
            xtp = [cp.tile([128, XW * C], f16, name=f"xtp{i}")
                   for i in range(4)]

            def lhsT_of(t):
                i, o = t // XW, t % XW
                return xtp[i][:, o * C : (o + 1) * C]

            bts = [bp_.tile([128, TPG * DPC], f8, name=f"btile{ci}",
                            tag=f"b{ci}") for ci in range(NCH)]
            # ring schedules: c0 leads scalar (it gates the first matmul),
            # cst trails (only the late bias seeds / epilogue need it);
            # x pieces slot into sync against the PE's tile deadlines
            sync_q = [("x", 0), ("c", 1), ("x", 1), ("c", 5), ("x", 2),
                      ("c", 7), ("c", 9), ("x", 3), ("c", 11), ("c", 13),
                      ("c", 15), ("c", 17)]
            scal_q = [("c", 0), ("c", 2), ("c", 3), ("c", 4), ("c", 6),
                      ("c", 8), ("c", 10), ("c", 12), ("c", 14), ("c", 16),
                      ("c", 18), ("c", 19), ("k", 0)]
            for eng, q in ((nc.sync, sync_q), (nc.scalar, scal_q)):
                for kind, i in q:
                    if kind == "x":
                        eng.dma_start(out=xtp[i][:],
                                      in_=xt_in[:, i * XW * C :
                                                (i + 1) * XW * C])
                    elif kind == "c":
                        eng.dma_start(out=bts[i][:],
                                      in_=b8_in[i * 128 : (i + 1) * 128, :])
                    else:
                        eng.dma_start(out=cst[:], in_=cst_in[:])

            # ---- stream B, accumulate agg[f, d] over s-tiles ----
            agg = pp.tile([128, DPC], f32, space="PSUM")
            for ch in range(NCH):
                b8_t = bts[ch]
                for k in range(TPG):
                    t = ch * TPG + k
                    segs = SEGS if t % 2 == 0 else SEGS[::-1]
                    for s0, s1 in segs:
                        nc.tensor.matmul(
                            out=agg[:, s0:s1],
                            lhsT=lhsT_of(t),
                            rhs=b8_t[:, k * DPC + s0 : k * DPC + s1],
                            start=(t == 0),
                            stop=(t == NST - 1),
                        )

            # bias pre-seed of the projection PSUM: fin[d, o] = b[o] + ...
            # (issued after the stream so the PE FIFO never blocks on cst)
            fin_all = pp2.tile([128, NDB * C], f32, space="PSUM")
            for bi in range(NDB):
                nc.tensor.matmul(
                    out=fin_all[:, bi * C : (bi + 1) * C],
                    lhsT=ones1[:], rhs=b_row[:], start=True, stop=False,
                )

            # ---- epilogue: scale+cast, project, add bias seed, store ----
            # 2-way split so the second half's DVE work overlaps the
            # first half's projections, and the two out DMAs ride
            # different rings
            agg16 = ep.tile([128, DPC], f16, tag="agg16")
            t2 = ep.tile([128, NDB * C], f16, tag="t2")
            H = DPC // 2
            for h in range(2):
                lo, hi = h * H, (h + 1) * H
                nc.vector.tensor_tensor(out=agg16[:, lo:hi],
                                        in0=agg[:, lo:hi],
                                        in1=disb_sb[:, lo:hi],
                                        op=mybir.AluOpType.mult)
                for bi in range(lo // C, hi // C):
                    nc.tensor.matmul(
                        out=fin_all[:, bi * C : (bi + 1) * C],
                        lhsT=agg16[:, bi * C : (bi + 1) * C],
                        rhs=wt_sb, start=False, stop=True,
                    )
                nc.vector.tensor_copy(out=t2[:, lo:hi], in_=fin_all[:, lo:hi])
                eng = nc.sync if h == 0 else nc.scalar
                eng.dma_start(out=out_t[:, lo:hi], in_=t2[:, lo:hi])
    nc.finalize()
    return nc


def kernel(x, edge_index, W, b):
    global last_exec_time_ns
    from concourse.bass_utils import run_bass_kernel_spmd

    x = np.ascontiguousarray(x, dtype=np.float32)
    edge_index = np.ascontiguousarray(edge_index, dtype=np.int32)
    W = np.ascontiguousarray(W, dtype=np.float32)
    b = np.ascontiguousarray(b, dtype=np.float32)

    xt, bs, dcols = _prep(x, edge_index)

    if "nc" not in _cache:
        _cache["nc"] = _build()
    nc = _cache["nc"]

    wt = W.T.astype(np.float16)                      # [f, o]
    in_maps = []
    for c in range(NCORES):
        cst = np.zeros((128, DPC + 2 * C), np.float16)
        cst[:, 0:DPC] = dcols[c].astype(np.float16)[None, :]
        cst[:, DPC : DPC + C] = wt
        cst[:, DPC + C : DPC + 2 * C] = b.astype(np.float16)[None, :]
        in_maps.append({
            "xt": xt,
            "b8": bs[c],
            "cst": np.ascontiguousarray(cst),
        })

    trace = os.environ.get("KERNEL_TRACE", "0") == "1"
    if trace:
        _install_ntff_shim()
    r = run_bass_kernel_spmd(
        nc, in_maps, core_ids=list(range(NCORES)), trace=trace,
        trace_cores=list(range(NCORES)) if trace else None,
    )
    last_exec_time_ns = r.exec_time_ns
    outs = []
    for c in range(NCORES):
        o = r.results[c]["out"].astype(np.float32)   # [128, NDB*C] tiled
        outs.append(o.reshape(128, NDB, C).transpose(1, 0, 2).reshape(DPC, C))
    out = np.concatenate(outs, axis=0)
    return np.ascontiguousarray(out[:N_NODES])


if __name__ == "__main__":
    rng = np.random.default_rng(0)
    x = rng.standard_normal((N_NODES, C)).astype(np.float32)
    ei = rng.integers(0, N_NODES, (2, N_EDGES)).astype(np.int32)
    W = rng.standard_normal((C, C)).astype(np.float32) * 0.1
    b = np.zeros(C, dtype=np.float32)
    out = kernel(x, ei, W, b)
    print("out", out.shape, out.dtype, float(np.abs(out).max()))
